# revision 5
# baseline (speedup 1.0000x reference)
"""Masked multi-head attention on 8 Trainium2 NeuronCores.

Problem: B=2, H=12, S=2048, D=64 attention with an int32 {0,1} mask
broadcast over heads.  out = softmax(mask ? QK^T/8 : -inf) @ V.

Sharding (8 cores, no cross-core comm):
  core c -> (b = c>>2, head-group hg = (c>>1)&1 -> 6 heads, q-half qh = c&1
  -> 1024 queries).  Each core computes full attention (all 2048 keys) for
  its 6 heads x 1024 queries.

Per-core device algorithm (matmul compute in fp16, fp32 accumulation):
  - scoresT[k, q] = K^T @ Q computed transposed so the probability matrix is
    produced directly in the [k (partitions), q (free)] layout the second
    matmul consumes.  The d=64 contraction uses PE row-tiling: two
    independent K=64 matmuls run concurrently in row groups (0,0)/(64,0).
  - softmax without max-subtraction (scores ~ N(0,1) after the 1/8 scale so
    exp cannot overflow), exp on ScalarE straight from PSUM with the 1/8
    scale fused, then probs *= mask (a {0,1} fp16 multiply on VectorE --
    mathematically identical to -inf masking; with S=2048 random bits a
    fully-masked row cannot occur).  ScalarE is the pacing engine: the
    whole kernel is one continuous stream of 96 exp instructions.
  - AV with [V | ones] (65 cols) as the stationary operand, streaming 512
    probs columns per matmul -> outT[d|sum, q] accumulates in PSUM; column
    65 collects the softmax denominator for free.  PE-transpose of the
    [65, 1024] result back to [q, d|sum], then out = t[:, 0:64] * (1/t[:, 64]).
  - int32->fp16 / f32->fp16 input casts run on the otherwise-idle GpSimd.
  - PE emission order interleaves head h+1's QK pairs with head h's AV
    chunks so the in-order PE stream always keeps ScalarE fed.
"""

import os
import sys

import numpy as np

for _p in ("/opt/trn_rl_repo",):
    if _p not in sys.path and os.path.isdir(_p):
        sys.path.insert(0, _p)

import concourse.bass as bass
import concourse.mybir as mybir
import concourse.tile as tile
from concourse import bacc
from concourse.bass_utils import run_bass_kernel_spmd
from concourse.masks import make_identity

FP16 = mybir.dt.float16
F32 = mybir.dt.float32
I32 = mybir.dt.int32

B, H, S, D = 2, 12, 2048, 64
NCORES = 8
HPC = 6        # heads per core
QPC = 1024     # queries per core
KT = S // 128  # 16 k-tiles
PAIRS = KT // 2
QTILES = QPC // 128

_NC_CACHE = None


def build_bass():
    """Build the single-core Bass/Tile program (SPMD across 8 cores)."""
    nc = bacc.Bacc("TRN2", target_bir_lowering=False, debug=False)

    qt = nc.declare_dram_parameter("qt", [HPC, D, QPC], F32, isOutput=False)
    kt = nc.declare_dram_parameter("kt", [HPC, D, S], F32, isOutput=False)
    v = nc.declare_dram_parameter("v", [HPC, S, D], F32, isOutput=False)
    maskt = nc.declare_dram_parameter("maskt", [S, QPC], I32, isOutput=False)
    o = nc.declare_dram_parameter("o", [HPC, QPC, D], F32, isOutput=True)

    with tile.TileContext(nc) as tc:
        with (
            tc.tile_pool(name="const", bufs=1) as const,
            tc.tile_pool(name="stage", bufs=2) as stage,
            tc.tile_pool(name="probs", bufs=16) as probs_pool,
            tc.tile_pool(name="avsb", bufs=2) as avsb_pool,
            tc.tile_pool(name="outp", bufs=4) as outp,
            tc.tile_pool(name="psc", bufs=2, space="PSUM") as psc,
            tc.tile_pool(name="ptp", bufs=2, space="PSUM") as ptp,
            tc.tile_pool(name="pav", bufs=1, space="PSUM") as pav,
        ):
            # Resident fp16 operands.
            # qh: Q^T per head, duplicated on partitions 0-63 / 64-127 so both
            #     PE row-groups can stream it.
            # kh: K^T per head "pair-stacked": rows 0-63 hold even k-tiles,
            #     rows 64-127 odd k-tiles, 128 columns per pair.
            # vt: [V | ones] per (head, k-tile).
            # mb: mask^T as fp16 {0,1}, [k-tile partition, k-tile idx, q].
            qh = const.tile([128, HPC, QPC], FP16)
            kh = const.tile([128, HPC, QPC], FP16)
            vt = const.tile([128, HPC, KT, 65], FP16)
            mb = const.tile([128, KT, QPC], FP16)
            ident = const.tile([65, 65], F32)
            make_identity(nc, ident[:])

            def load_head(h):
                q_stage = stage.tile([64, QPC], F32, tag="qs")
                nc.sync.dma_start(q_stage[:], qt[h])
                nc.gpsimd.tensor_copy(qh[0:64, h, :], q_stage[:])
                nc.sync.dma_start(qh[64:128, h, :], qh[0:64, h, :])

                k_stage = stage.tile([128, QPC], F32, tag="ks")
                kview = kt[h].rearrange("d (a two c) -> d a two c", two=2, c=128)
                nc.sync.dma_start(
                    k_stage[0:64, :].rearrange("d (a c) -> d a c", c=128),
                    kview[:, :, 0, :],
                )
                nc.sync.dma_start(
                    k_stage[64:128, :].rearrange("d (a c) -> d a c", c=128),
                    kview[:, :, 1, :],
                )
                nc.gpsimd.tensor_copy(kh[:, h, :], k_stage[:])

                v_stage = stage.tile([128, KT, D], F32, tag="vs")
                nc.sync.dma_start(v_stage[:], v[h].rearrange("(t p) c -> p t c", p=128))
                nc.gpsimd.memset(vt[:, h, :, :], 1.0)
                nc.gpsimd.tensor_copy(vt[:, h, :, 0:64], v_stage[:])

            def qk_pair(h, j):
                """QK^T + exp + mask for head h, k-tile pair (2j, 2j+1)."""
                pr = probs_pool.tile([128, 2 * QPC], FP16, tag="pp")
                for r in range(2):  # k-tiles 2j (rows 0-63), 2j+1 (64-127)
                    lo, hi = 64 * r, 64 * r + 64
                    sc = psc.tile([128, QPC], F32, tag="sc")
                    for qc in range(QPC // 512):
                        nc.tensor.matmul(
                            sc[:, qc * 512 : qc * 512 + 512],
                            kh[lo:hi, h, 128 * j : 128 * j + 128],
                            qh[lo:hi, h, qc * 512 : qc * 512 + 512],
                            start=True,
                            stop=True,
                            tile_position=(64 * r, 0),
                        )
                    if h == 0:
                        t = 2 * j + r
                        m_stage = stage.tile([128, QPC], I32, tag="ms")
                        nc.sync.dma_start(m_stage[:], maskt[128 * t : 128 * t + 128, :])
                        nc.gpsimd.tensor_copy(mb[:, t, :], m_stage[:])
                    nc.scalar.activation(
                        pr[:, r * QPC : (r + 1) * QPC],
                        sc[:],
                        mybir.ActivationFunctionType.Exp,
                        scale=0.125,
                    )
                nc.vector.tensor_mul(
                    pr.rearrange("p (t q) -> p t q", t=2),
                    pr.rearrange("p (t q) -> p t q", t=2),
                    mb[:, 2 * j : 2 * j + 2, :],
                )
                return pr

            def av_chunk(h, avp, pairs, j):
                """Accumulate k-tiles 2j, 2j+1 of head h into avp [65, QPC]."""
                for r in range(2):
                    k = 2 * j + r
                    for qc in range(QPC // 512):
                        nc.tensor.matmul(
                            avp[:, qc * 512 : qc * 512 + 512],
                            vt[:, h, k, :],
                            pairs[j][:, r * QPC + qc * 512 : r * QPC + qc * 512 + 512],
                            start=(k == 0),
                            stop=(k == KT - 1),
                        )

            def av_epilogue(h, avp):
                av_sb = avsb_pool.tile([65, QPC], F32, tag="avsb")
                nc.vector.tensor_copy(av_sb[:], avp[:])
                for s in range(QTILES):
                    tp = ptp.tile([128, 65], F32, tag="tp")
                    nc.tensor.transpose(
                        tp[:], av_sb[:, 128 * s : 128 * s + 128], ident[:]
                    )
                    rec = outp.tile([128, 1], F32, tag="rec")
                    nc.vector.reciprocal(rec[:], tp[:, 64:65])
                    osb = outp.tile([128, D], F32, tag="os")
                    nc.vector.tensor_scalar_mul(osb[:], tp[:, 0:64], rec[:])
                    nc.sync.dma_start(o[h, 128 * s : 128 * s + 128, :], osb[:])

            # Prologue: head 0's QK/exp/mask stream.
            load_head(0)
            pairs_prev = [qk_pair(0, j) for j in range(PAIRS)]

            # Steady state: interleave head h+1's QK pairs (feeding ScalarE)
            # with head h's AV chunks in the in-order PE stream.
            for h in range(HPC):
                last = h == HPC - 1
                if not last:
                    load_head(h + 1)
                avp = pav.tile([65, QPC], F32, tag="av")
                pairs_cur = []
                for j in range(PAIRS):
                    if not last:
                        pairs_cur.append(qk_pair(h + 1, j))
                    av_chunk(h, avp, pairs_prev, j)
                av_epilogue(h, avp)
                pairs_prev = pairs_cur

    nc.compile()
    return nc


def _shard(c, Q, K, V, mask):
    b, hg, qh = c >> 2, (c >> 1) & 1, c & 1
    hs = slice(hg * HPC, hg * HPC + HPC)
    qs = slice(qh * QPC, qh * QPC + QPC)
    return {
        "qt": np.ascontiguousarray(Q[b, hs, qs, :].transpose(0, 2, 1)),
        "kt": np.ascontiguousarray(K[b, hs, :, :].transpose(0, 2, 1)),
        "v": np.ascontiguousarray(V[b, hs, :, :]),
        "maskt": np.ascontiguousarray(mask[b, 0, qs, :].T),
    }


def get_nc():
    global _NC_CACHE
    if _NC_CACHE is None:
        _NC_CACHE = build_bass()
    return _NC_CACHE


def kernel(Q, K, V, mask):
    Q = np.asarray(Q, dtype=np.float32)
    K = np.asarray(K, dtype=np.float32)
    V = np.asarray(V, dtype=np.float32)
    mask = np.asarray(mask, dtype=np.int32)

    in_maps = [_shard(c, Q, K, V, mask) for c in range(NCORES)]
    res = run_bass_kernel_spmd(get_nc(), in_maps, list(range(NCORES))).results

    out = np.empty((B, H, S, D), dtype=np.float32)
    for c in range(NCORES):
        b, hg, qh = c >> 2, (c >> 1) & 1, c & 1
        out[b, hg * HPC : hg * HPC + HPC, qh * QPC : qh * QPC + QPC, :] = res[c]["o"]
    return out


# revision 7
# speedup vs baseline: 1.0357x; 1.0357x over previous
"""Masked multi-head attention on 8 Trainium2 NeuronCores.

Problem: B=2, H=12, S=2048, D=64 attention with an int32 {0,1} mask
broadcast over heads.  out = softmax(mask ? QK^T/8 : -inf) @ V.

Sharding (8 cores, no cross-core comm):
  core c -> (b = c>>2, head-group hg = (c>>1)&1 -> 6 heads, q-half qh = c&1
  -> 1024 queries).  Each core computes full attention (all 2048 keys) for
  its 6 heads x 1024 queries.

Per-core device algorithm (matmul compute in fp16, fp32 accumulation):
  - scoresT[k, q] = K^T @ Q computed transposed so the probability matrix is
    produced directly in the [k (partitions), q (free)] layout the second
    matmul consumes.  The d=64 contraction uses PE row-tiling: two
    independent K=64 matmuls run concurrently in row groups (0,0)/(64,0).
  - softmax without max-subtraction (scores ~ N(0,1) after the 1/8 scale so
    exp cannot overflow), exp on ScalarE straight from PSUM with the 1/8
    scale fused, then probs *= mask (a {0,1} fp16 multiply on VectorE --
    mathematically identical to -inf masking; with S=2048 random bits a
    fully-masked row cannot occur).  ScalarE paces the kernel: one
    continuous stream of 96 exp instructions.
  - AV with [V | ones | zero-pad] (80 cols, padded so the DMA xbar
    transpose's 16-row granularity is satisfied) as the stationary operand, streaming 512
    probs columns per matmul -> outT[d|sum, q] accumulates in PSUM; column
    65 collects the softmax denominator for free.  The [65, 1024] result is
    transposed back to [q, d|sum] with the DMA xbar (SBUF->SBUF, fp16),
    then out = t[:, 0:64] * (1 / t[:, 64]).
  - Emission interleaves head h+1's QK quads with head h's AV chunks in the
    in-order PE stream, loads inputs three heads ahead, and casts two heads
    ahead, so neither ScalarE nor VectorE ever stalls on a DMA.
"""

import os
import sys

import numpy as np

for _p in ("/opt/trn_rl_repo",):
    if _p not in sys.path and os.path.isdir(_p):
        sys.path.insert(0, _p)

import concourse.bass as bass
import concourse.mybir as mybir
import concourse.tile as tile
from concourse import bacc
from concourse.bass_utils import run_bass_kernel_spmd

FP16 = mybir.dt.float16
F32 = mybir.dt.float32
I32 = mybir.dt.int32

B, H, S, D = 2, 12, 2048, 64
NCORES = 8
HPC = 6         # heads per core
QPC = 1024      # queries per core
KT = S // 128   # 16 k-tiles
QUADS = KT // 4  # 4 quads of 4 k-tiles
QTILES = QPC // 128

_NC_CACHE = None


def build_bass():
    """Build the single-core Bass/Tile program (SPMD across 8 cores)."""
    nc = bacc.Bacc("TRN2", target_bir_lowering=False, debug=False)

    qt = nc.declare_dram_parameter("qt", [HPC, D, QPC], F32, isOutput=False)
    kt = nc.declare_dram_parameter("kt", [HPC, D, S], F32, isOutput=False)
    v = nc.declare_dram_parameter("v", [HPC, S, D], F32, isOutput=False)
    maskt = nc.declare_dram_parameter("maskt", [S, QPC], I32, isOutput=False)
    o = nc.declare_dram_parameter("o", [HPC, QPC, D], F32, isOutput=True)

    with tile.TileContext(nc) as tc:
        with (
            tc.tile_pool(name="const", bufs=1) as const,
            tc.tile_pool(name="stage", bufs=3) as stage,
            tc.tile_pool(name="mstage", bufs=2) as mstage,
            tc.tile_pool(name="probs", bufs=8) as probs_pool,
            tc.tile_pool(name="avsb", bufs=2) as avsb_pool,
            tc.tile_pool(name="outp", bufs=4) as outp,
            tc.tile_pool(name="psc", bufs=3, space="PSUM") as psc,
            tc.tile_pool(name="pav", bufs=1, space="PSUM") as pav,
        ):
            # Resident fp16 operands.
            # qh: Q^T per head, duplicated on partitions 0-63 / 64-127 so both
            #     PE row-groups can stream it.
            # kh: K^T per head "pair-stacked": rows 0-63 hold even k-tiles,
            #     rows 64-127 odd k-tiles, 128 columns per pair.
            # vt: [V | ones] per (head, k-tile).
            # mb: mask^T as fp16 {0,1}, [k-tile partition, k-tile idx, q].
            qh = const.tile([128, HPC, QPC], FP16)
            kh = const.tile([128, HPC, QPC], FP16)
            vt = const.tile([128, HPC, KT, 80], FP16)
            mb = const.tile([128, KT, QPC], FP16)

            stages = {}

            def load_dmas(h):
                q_stage = stage.tile([64, QPC], F32, tag="qs")
                nc.sync.dma_start(q_stage[:], qt[h])

                k_stage = stage.tile([128, QPC], F32, tag="ks")
                kview = kt[h].rearrange("d (a two c) -> d a two c", two=2, c=128)
                nc.sync.dma_start(
                    k_stage[0:64, :].rearrange("d (a c) -> d a c", c=128),
                    kview[:, :, 0, :],
                )
                nc.sync.dma_start(
                    k_stage[64:128, :].rearrange("d (a c) -> d a c", c=128),
                    kview[:, :, 1, :],
                )

                v_stage = stage.tile([128, KT, D], F32, tag="vs")
                nc.sync.dma_start(v_stage[:], v[h].rearrange("(t p) c -> p t c", p=128))
                stages[h] = (q_stage, k_stage, v_stage)

            def casts(h):
                q_stage, k_stage, v_stage = stages.pop(h)
                nc.vector.tensor_copy(qh[0:64, h, :], q_stage[:])
                nc.sync.dma_start(qh[64:128, h, :], qh[0:64, h, :])
                nc.vector.tensor_copy(kh[:, h, :], k_stage[:])
                nc.gpsimd.memset(vt[:, h, :, :], 0.0)
                nc.gpsimd.memset(vt[:, h, :, 64:65], 1.0)
                nc.vector.tensor_copy(vt[:, h, :, 0:64], v_stage[:])

            def qk_quad(h, i):
                """QK^T + exp + mask for head h, k-tiles 4i..4i+3."""
                pr = probs_pool.tile([128, 4 * QPC], FP16, tag="pp")
                for j in (2 * i, 2 * i + 1):  # k-tile pairs
                    for r in range(2):        # k-tile 2j+r in PE row group r
                        lo, hi = 64 * r, 64 * r + 64
                        t = 2 * j + r
                        sc = psc.tile([128, QPC], F32, tag="sc")
                        for qc in range(QPC // 512):
                            nc.tensor.matmul(
                                sc[:, qc * 512 : qc * 512 + 512],
                                kh[lo:hi, h, 128 * j : 128 * j + 128],
                                qh[lo:hi, h, qc * 512 : qc * 512 + 512],
                                start=True,
                                stop=True,
                                tile_position=(64 * r, 0),
                            )
                        if h == 0:
                            m_stage = mstage.tile([128, QPC], I32, tag="ms")
                            nc.sync.dma_start(
                                m_stage[:], maskt[128 * t : 128 * t + 128, :]
                            )
                            nc.vector.tensor_copy(mb[:, t, :], m_stage[:])
                        nc.scalar.activation(
                            pr[:, (t - 4 * i) * QPC : (t - 4 * i + 1) * QPC],
                            sc[:],
                            mybir.ActivationFunctionType.Exp,
                            scale=0.125,
                        )
                nc.vector.tensor_mul(
                    pr.rearrange("p (t q) -> p t q", t=4),
                    pr.rearrange("p (t q) -> p t q", t=4),
                    mb[:, 4 * i : 4 * i + 4, :],
                )
                return pr

            def av_chunk(h, avp, quads, i):
                """Accumulate k-tiles 4i..4i+3 of head h into avp [65, QPC]."""
                for u in range(4):
                    k = 4 * i + u
                    for qc in range(QPC // 512):
                        nc.tensor.matmul(
                            avp[:, qc * 512 : qc * 512 + 512],
                            vt[:, h, k, :],
                            quads[i][:, u * QPC + qc * 512 : u * QPC + qc * 512 + 512],
                            start=(k == 0),
                            stop=(k == KT - 1),
                        )

            def av_epilogue(h, avp):
                av_sb = avsb_pool.tile([80, QPC], FP16, tag="avsb")
                nc.vector.tensor_copy(av_sb[:], avp[:])
                for s in range(QTILES):
                    tp = outp.tile([128, 80], FP16, tag="tp")
                    nc.sync.dma_start_transpose(
                        tp[:], av_sb[:, 128 * s : 128 * s + 128]
                    )
                    rec = outp.tile([128, 1], F32, tag="rec")
                    nc.vector.reciprocal(rec[:], tp[:, 64:65])
                    osb = outp.tile([128, D], F32, tag="os")
                    nc.vector.tensor_scalar_mul(osb[:], tp[:, 0:64], rec[:])
                    nc.gpsimd.dma_start(o[h, 128 * s : 128 * s + 128, :], osb[:])

            # Prologue: warm the load/cast pipeline, then head 0's QK stream.
            load_dmas(0)
            load_dmas(1)
            casts(0)
            quads_prev = [qk_quad(0, i) for i in range(QUADS)]
            load_dmas(2)
            casts(1)

            # Steady state: head h+1's QK quads interleaved with head h's AV.
            for h in range(HPC):
                if h + 3 < HPC:
                    load_dmas(h + 3)
                if h + 2 < HPC:
                    casts(h + 2)
                avp = pav.tile([80, QPC], F32, tag="av")
                quads_cur = []
                for i in range(QUADS):
                    if h + 1 < HPC:
                        quads_cur.append(qk_quad(h + 1, i))
                    av_chunk(h, avp, quads_prev, i)
                av_epilogue(h, avp)
                quads_prev = quads_cur

    nc.compile()
    return nc


def _shard(c, Q, K, V, mask):
    b, hg, qh = c >> 2, (c >> 1) & 1, c & 1
    hs = slice(hg * HPC, hg * HPC + HPC)
    qs = slice(qh * QPC, qh * QPC + QPC)
    return {
        "qt": np.ascontiguousarray(Q[b, hs, qs, :].transpose(0, 2, 1)),
        "kt": np.ascontiguousarray(K[b, hs, :, :].transpose(0, 2, 1)),
        "v": np.ascontiguousarray(V[b, hs, :, :]),
        "maskt": np.ascontiguousarray(mask[b, 0, qs, :].T),
    }


def get_nc():
    global _NC_CACHE
    if _NC_CACHE is None:
        _NC_CACHE = build_bass()
    return _NC_CACHE


def kernel(Q, K, V, mask):
    Q = np.asarray(Q, dtype=np.float32)
    K = np.asarray(K, dtype=np.float32)
    V = np.asarray(V, dtype=np.float32)
    mask = np.asarray(mask, dtype=np.int32)

    in_maps = [_shard(c, Q, K, V, mask) for c in range(NCORES)]
    res = run_bass_kernel_spmd(get_nc(), in_maps, list(range(NCORES))).results

    out = np.empty((B, H, S, D), dtype=np.float32)
    for c in range(NCORES):
        b, hg, qh = c >> 2, (c >> 1) & 1, c & 1
        out[b, hg * HPC : hg * HPC + HPC, qh * QPC : qh * QPC + QPC, :] = res[c]["o"]
    return out


# revision 9
# speedup vs baseline: 1.3189x; 1.2735x over previous
"""Masked multi-head attention on 8 Trainium2 NeuronCores.

Problem: B=2, H=12, S=2048, D=64 attention with an int32 {0,1} mask
broadcast over heads.  out = softmax(mask ? QK^T/8 : -inf) @ V.

Sharding (8 cores, no cross-core comm):
  core c -> (b = c>>2, head-group hg = (c>>1)&1 -> 6 heads, q-half qh = c&1
  -> 1024 queries).  Each core computes full attention (all 2048 keys) for
  its 6 heads x 1024 queries.

Per-core device algorithm (matmul compute in fp16, fp32 accumulation):
  - scoresT[k, q] = K^T @ Q computed transposed so the probability matrix is
    produced directly in the [k (partitions), q (free)] layout the second
    matmul consumes.  The d=64 contraction uses PE row-tiling: two
    independent K=64 matmuls run concurrently in row groups (0,0)/(64,0).
  - softmax without max-subtraction (scores ~ N(0,1) after the 1/8 scale so
    exp cannot overflow), exp on ScalarE straight from PSUM with the 1/8
    scale fused, then probs *= mask (a {0,1} fp16 multiply on VectorE --
    mathematically identical to -inf masking; with S=2048 random bits a
    fully-masked row cannot occur).  ScalarE paces the kernel: one
    continuous stream of 96 exp instructions.
  - AV with [V | ones | zero-pad] (80 cols) as the stationary operand,
    streaming 512 probs columns per matmul -> outT[d|sum, q] accumulates in
    PSUM in two q-phases of [80, 512]; column 64 collects the softmax
    denominator for free.  PE-transposes bring the result back to
    [q, d|sum], then out = t[:, 0:64] * (1 / t[:, 64]).
  - Emission interleaves head h+1's QK quads with head h's AV chunks in the
    in-order PE stream, loads inputs three heads ahead, and casts two heads
    ahead, so neither ScalarE nor VectorE ever stalls on a DMA.
"""

import os
import sys

import numpy as np

for _p in ("/opt/trn_rl_repo",):
    if _p not in sys.path and os.path.isdir(_p):
        sys.path.insert(0, _p)

import concourse.bass as bass
import concourse.mybir as mybir
import concourse.tile as tile
from concourse import bacc
from concourse.bass_utils import run_bass_kernel_spmd
from concourse.masks import make_identity

FP16 = mybir.dt.float16
F32 = mybir.dt.float32
I32 = mybir.dt.int32

B, H, S, D = 2, 12, 2048, 64
NCORES = 8
HPC = 6         # heads per core
QPC = 1024      # queries per core
KT = S // 128   # 16 k-tiles
QUADS = KT // 4  # 4 quads of 4 k-tiles
QTILES = QPC // 128

_NC_CACHE = None


def build_bass():
    """Build the single-core Bass/Tile program (SPMD across 8 cores)."""
    nc = bacc.Bacc("TRN2", target_bir_lowering=False, debug=False)

    qt = nc.declare_dram_parameter("qt", [HPC, D, QPC], F32, isOutput=False)
    kt = nc.declare_dram_parameter("kt", [HPC, D, S], F32, isOutput=False)
    v = nc.declare_dram_parameter("v", [HPC, S, D], F32, isOutput=False)
    maskt = nc.declare_dram_parameter("maskt", [S, QPC], I32, isOutput=False)
    o = nc.declare_dram_parameter("o", [HPC, QPC, D], F32, isOutput=True)

    with tile.TileContext(nc) as tc:
        with (
            tc.tile_pool(name="const", bufs=1) as const,
            tc.tile_pool(name="stage", bufs=3) as stage,
            tc.tile_pool(name="mstage", bufs=2) as mstage,
            tc.tile_pool(name="probs", bufs=8) as probs_pool,
            tc.tile_pool(name="avsb", bufs=4) as avsb_pool,
            tc.tile_pool(name="outp", bufs=16) as outp,
            tc.tile_pool(name="psc", bufs=3, space="PSUM") as psc,
            tc.tile_pool(name="pav", bufs=1, space="PSUM") as pav,
            tc.tile_pool(name="ptp", bufs=1, space="PSUM") as ptp,
        ):
            # Resident fp16 operands.
            # qh: Q^T per head, duplicated on partitions 0-63 / 64-127 so both
            #     PE row-groups can stream it.
            # kh: K^T per head "pair-stacked": rows 0-63 hold even k-tiles,
            #     rows 64-127 odd k-tiles, 128 columns per pair.
            # vt: [V | ones] per (head, k-tile).
            # mb: mask^T as fp16 {0,1}, [k-tile partition, k-tile idx, q].
            qh = const.tile([128, HPC, QPC], FP16)
            kh = const.tile([128, HPC, QPC], FP16)
            vt = const.tile([128, HPC, KT, 80], FP16)
            mb = const.tile([128, KT, QPC], FP16)
            ident = const.tile([80, 80], FP16)
            make_identity(nc, ident[:])

            stages = {}

            def load_dmas(h):
                q_stage = stage.tile([64, QPC], F32, tag="qs")
                nc.sync.dma_start(q_stage[:], qt[h])

                k_stage = stage.tile([128, QPC], F32, tag="ks")
                kview = kt[h].rearrange("d (a two c) -> d a two c", two=2, c=128)
                nc.sync.dma_start(
                    k_stage[0:64, :].rearrange("d (a c) -> d a c", c=128),
                    kview[:, :, 0, :],
                )
                nc.sync.dma_start(
                    k_stage[64:128, :].rearrange("d (a c) -> d a c", c=128),
                    kview[:, :, 1, :],
                )

                v_stage = stage.tile([128, KT, D], F32, tag="vs")
                nc.sync.dma_start(v_stage[:], v[h].rearrange("(t p) c -> p t c", p=128))
                stages[h] = (q_stage, k_stage, v_stage)

            def casts(h):
                q_stage, k_stage, v_stage = stages.pop(h)
                nc.vector.tensor_copy(qh[0:64, h, :], q_stage[:])
                nc.sync.dma_start(qh[64:128, h, :], qh[0:64, h, :])
                nc.vector.tensor_copy(kh[:, h, :], k_stage[:])
                nc.gpsimd.memset(vt[:, h, :, :], 0.0)
                nc.gpsimd.memset(vt[:, h, :, 64:65], 1.0)
                nc.vector.tensor_copy(vt[:, h, :, 0:64], v_stage[:])

            def qk_quad(h, i):
                """QK^T + exp + mask for head h, k-tiles 4i..4i+3."""
                pr = probs_pool.tile([128, 4 * QPC], FP16, tag="pp")
                for j in (2 * i, 2 * i + 1):  # k-tile pairs
                    for r in range(2):        # k-tile 2j+r in PE row group r
                        lo, hi = 64 * r, 64 * r + 64
                        t = 2 * j + r
                        sc = psc.tile([128, QPC], F32, tag="sc")
                        for qc in range(QPC // 512):
                            nc.tensor.matmul(
                                sc[:, qc * 512 : qc * 512 + 512],
                                kh[lo:hi, h, 128 * j : 128 * j + 128],
                                qh[lo:hi, h, qc * 512 : qc * 512 + 512],
                                start=True,
                                stop=True,
                                tile_position=(64 * r, 0),
                            )
                        if h == 0:
                            m_stage = mstage.tile([128, QPC], I32, tag="ms")
                            nc.sync.dma_start(
                                m_stage[:], maskt[128 * t : 128 * t + 128, :]
                            )
                            nc.vector.tensor_copy(mb[:, t, :], m_stage[:])
                        nc.scalar.activation(
                            pr[:, (t - 4 * i) * QPC : (t - 4 * i + 1) * QPC],
                            sc[:],
                            mybir.ActivationFunctionType.Exp,
                            scale=0.125,
                        )
                nc.vector.tensor_mul(
                    pr.rearrange("p (t q) -> p t q", t=4),
                    pr.rearrange("p (t q) -> p t q", t=4),
                    mb[:, 4 * i : 4 * i + 4, :],
                )
                return pr

            def av_part(h, avp, quads, i):
                """AV for head h, 8 of 32 (qc, k-tile) steps per quad slot i.

                i = 0,1 accumulate phase qc=0 (k-tiles 8i..8i+7 of quads),
                i = 2,3 accumulate phase qc=1.  All quads of head h are
                complete by the time section h runs.
                """
                qc = i // 2
                for u in range(8):
                    k = 8 * (i % 2) + u
                    nc.tensor.matmul(
                        avp[:],
                        vt[:, h, k, :],
                        quads[k // 4][:, (k % 4) * QPC + qc * 512 : (k % 4) * QPC + qc * 512 + 512],
                        start=(k == 0),
                        stop=(k == KT - 1),
                    )

            av_sbs = {}

            def av_copy(h, qc, avp):
                av_sb = avsb_pool.tile([80, 512], FP16, tag="avsb")
                nc.vector.tensor_copy(av_sb[:], avp[:])
                av_sbs[(h, qc)] = av_sb

            def transpose_epilogue(h, s):
                """Output q-tile s (of 8) for head h: transpose + divide."""
                av_sb = av_sbs[(h, s // 4)]
                tp = ptp.tile([128, 80], FP16, tag="tp")
                nc.tensor.transpose(
                    tp[:], av_sb[:, 128 * (s % 4) : 128 * (s % 4) + 128], ident[:]
                )
                rec = outp.tile([128, 1], F32, tag="rec")
                nc.vector.reciprocal(rec[:], tp[:, 64:65])
                osb = outp.tile([128, D], F32, tag="os")
                nc.vector.tensor_scalar_mul(osb[:], tp[:, 0:64], rec[:])
                nc.gpsimd.dma_start(o[h, 128 * s : 128 * s + 128, :], osb[:])

            # Prologue: warm the load/cast pipeline, then head 0's QK stream.
            load_dmas(0)
            casts(0)
            load_dmas(1)
            quads_prev = [qk_quad(0, i) for i in range(QUADS)]
            load_dmas(2)
            casts(1)

            # Steady state section h: head h+1's QK quads (feeding ScalarE)
            # interleaved with head h's AV and head h-1's transpose/divide
            # epilogue, all in dependency-satisfied order so no in-order
            # engine stream stalls at a head boundary.
            for h in range(HPC):
                if h + 3 < HPC:
                    load_dmas(h + 3)
                if h + 2 < HPC:
                    casts(h + 2)
                avp = None
                quads_cur = []
                for i in range(QUADS):
                    if h + 1 < HPC:
                        quads_cur.append(qk_quad(h + 1, i))
                    if i % 2 == 0:
                        avp = pav.tile([80, 512], F32, tag="av")
                    av_part(h, avp, quads_prev, i)
                    if i % 2 == 1:
                        av_copy(h, i // 2, avp)
                    if h >= 1:
                        transpose_epilogue(h - 1, 2 * i)
                        transpose_epilogue(h - 1, 2 * i + 1)
                quads_prev = quads_cur
            for s in range(2 * QUADS):
                transpose_epilogue(HPC - 1, s)

    nc.compile()
    return nc


def _shard(c, Q, K, V, mask):
    b, hg, qh = c >> 2, (c >> 1) & 1, c & 1
    hs = slice(hg * HPC, hg * HPC + HPC)
    qs = slice(qh * QPC, qh * QPC + QPC)
    return {
        "qt": np.ascontiguousarray(Q[b, hs, qs, :].transpose(0, 2, 1)),
        "kt": np.ascontiguousarray(K[b, hs, :, :].transpose(0, 2, 1)),
        "v": np.ascontiguousarray(V[b, hs, :, :]),
        "maskt": np.ascontiguousarray(mask[b, 0, qs, :].T),
    }


def get_nc():
    global _NC_CACHE
    if _NC_CACHE is None:
        _NC_CACHE = build_bass()
    return _NC_CACHE


def kernel(Q, K, V, mask):
    Q = np.asarray(Q, dtype=np.float32)
    K = np.asarray(K, dtype=np.float32)
    V = np.asarray(V, dtype=np.float32)
    mask = np.asarray(mask, dtype=np.int32)

    in_maps = [_shard(c, Q, K, V, mask) for c in range(NCORES)]
    res = run_bass_kernel_spmd(get_nc(), in_maps, list(range(NCORES))).results

    out = np.empty((B, H, S, D), dtype=np.float32)
    for c in range(NCORES):
        b, hg, qh = c >> 2, (c >> 1) & 1, c & 1
        out[b, hg * HPC : hg * HPC + HPC, qh * QPC : qh * QPC + QPC, :] = res[c]["o"]
    return out
